# revision 14
# baseline (speedup 1.0000x reference)
"""Trainium2 Bass kernel for Expansion + CPSDropout (bf16 output).

Computes, for x[4,256,64,64] f32 and rand_vals[320,320] f32:
    out[b,c,5i+p,5j+q] = xpad[b,c,i+p,j+q] * M[5i+p,5j+q]
    M = (rand_vals > 0.25, forced True at [2::5,2::5]) * (4/3)

Strategy (8 cores, data parallel over the 1024 (b,c) channels, 128/core):
  - host: x*(4/3) -> bf16; binary mask {0,1} bf16 with zeros at border
    positions whose source reads fall in the zero padding.
  - device: x staged into guard-padded xg[128, 4360]; each i-tile's 1600
    outputs come from ONE DVE tensor_tensor with input AP
    out[a,j,q] = xg[64*(i+a) + j + q] * mask[a*320+5j+q].
  - mask replication across partitions via PE broadcast (ones^T @ mask)
    into [128,2048] PSUM tiles (512-col matmuls, bank aligned).
    per group of 4 i-tiles, 3 are evacuated PSUM->SBUF bf16 by ScalarE
    (DVE multiplies from SBUF, faster mode) and 1 is multiplied by the
    DVE straight from PSUM — balancing ScalarE/DVE against the PE-paced
    2-deep PSUM ring.
  - output bf16, host upcasts to f32 (rel err ~2e-3 << 2e-2 gate).
"""

import numpy as np
import ml_dtypes

import concourse.bass as bass
import concourse.bacc as bacc
import concourse.mybir as mybir
import concourse.tile as tile
from concourse.bass_utils import run_bass_kernel_spmd

P = 128
N_CORES = 8
H = W = 64
S = 5
S2 = S // 2
OUT_HW = H * S            # 320
OUT_ELEMS = OUT_HW * OUT_HW
RATE = 0.25

XG_F = 68 * W + 8         # 4360
XG_X0 = 2 * W + 2         # 130

I_PER_G = 4               # i-tiles per store group
GROUPS = H // I_PER_G     # 8
TILE_F = S * OUT_HW       # 1600
G_F = I_PER_G * TILE_F    # 12800

TILE_PATTERN = "AAAA"  # all tiles evac'd; evac split to free PSUM early

_CACHE = {}


def _build_nc():
    assert len(TILE_PATTERN) == I_PER_G
    nc = bacc.Bacc("TRN2", target_bir_lowering=False)
    x_t = nc.dram_tensor("x", [P, H * W], mybir.dt.bfloat16, kind="ExternalInput")
    m_t = nc.dram_tensor("mask", [GROUPS, G_F], mybir.dt.bfloat16, kind="ExternalInput")
    o_t = nc.dram_tensor("out", [P, OUT_ELEMS], mybir.dt.bfloat16,
                         kind="ExternalOutput")

    with tile.TileContext(nc) as tc:
        with (
            tc.tile_pool(name="const", bufs=1) as constp,
            tc.tile_pool(name="xbuf", bufs=1) as xbufp,
            tc.tile_pool(name="mstage", bufs=4) as mstp,
            tc.tile_pool(name="msb", bufs=2) as msbp,
            tc.tile_pool(name="obuf", bufs=3) as obufp,
            tc.tile_pool(name="mpsum", bufs=2, space="PSUM") as psump,
        ):
            ones_bf = constp.tile([1, P], mybir.dt.bfloat16)
            nc.vector.memset(ones_bf[:], 1.0)

            xg = xbufp.tile([P, XG_F], mybir.dt.bfloat16)
            # only the guard strips outside x need to be finite (borders are
            # killed by host-side mask zeros); keep ScalarE free
            nc.vector.memset(xg[:, 0:XG_X0], 0.0)
            nc.vector.memset(xg[:, XG_X0 + H * W : XG_F], 0.0)
            nc.gpsimd.dma_start(out=xg[:, XG_X0 : XG_X0 + H * W], in_=x_t[:])

            xg_ap = xg[:]
            xg_pdim = list(xg_ap.ap[0])
            TT_DIMS = [[OUT_HW, S], [S, H], [1, S]]          # (a, j, q) dense
            TT_DIMS2 = [[2 * OUT_HW, S], [2 * S, H], [2, S]]  # stride-2 bf16 view

            def tt(i, out_tile, out_off, in1_tensor, in1_offset, in1_pdim,
                   dims=TT_DIMS):
                in0 = bass.AP(
                    tensor=xg_ap.tensor,
                    offset=xg_ap.offset + i * W,
                    ap=[xg_pdim, [W, S], [1, H], [1, S]],
                )
                o1 = bass.AP(
                    tensor=out_tile.tensor,
                    offset=out_tile.offset + out_off,
                    ap=[list(out_tile.ap[0])] + TT_DIMS,
                )
                i1 = bass.AP(
                    tensor=in1_tensor, offset=in1_offset,
                    ap=[in1_pdim] + dims,
                )
                nc.vector.tensor_tensor(
                    out=o1, in0=in0, in1=i1, op=mybir.AluOpType.mult
                )

            for g in range(GROUPS):
                obuf = obufp.tile([P, G_F], mybir.dt.bfloat16)
                mst = mstp.tile([1, G_F], mybir.dt.bfloat16)
                nc.gpsimd.dma_start(out=mst[:], in_=m_t[g : g + 1, :])

                msb = msbp.tile([P, G_F], mybir.dt.bfloat16)
                msb_ap = msb[:]
                for u in range(I_PER_G):
                    path = TILE_PATTERN[u]
                    i = g * I_PER_G + u
                    ps = psump.tile([P, TILE_F], mybir.dt.float32)
                    for j0, j1 in ((0, 512), (512, 1024), (1024, 1536),
                                   (1536, 1600)):
                        nc.tensor.matmul(
                            ps[:, j0:j1],
                            ones_bf[:],
                            mst[0:1, u * TILE_F + j0 : u * TILE_F + j1],
                            start=True, stop=True,
                        )
                    if path == "A":
                        # split evac at matmul boundaries: starts after mm1,
                        # frees the PSUM slot right after mm3
                        nc.scalar.copy(
                            out=msb[:, u * TILE_F : u * TILE_F + 1024],
                            in_=ps[:, 0:1024],
                        )
                        nc.scalar.copy(
                            out=msb[:, u * TILE_F + 1024 : (u + 1) * TILE_F],
                            in_=ps[:, 1024:1600],
                        )
                        tt(i, obuf[:], u * TILE_F,
                           msb_ap.tensor, msb_ap.offset + u * TILE_F,
                           list(msb_ap.ap[0]))
                    else:
                        ps_ap = ps[:]
                        tt(i, obuf[:], u * TILE_F,
                           ps_ap.tensor, ps_ap.offset, list(ps_ap.ap[0]))
                nc.sync.dma_start(out=o_t[:, g * G_F : (g + 1) * G_F], in_=obuf[:])
    nc.compile()
    return nc


def _get_nc():
    if "nc" not in _CACHE:
        _CACHE["nc"] = _build_nc()
    return _CACHE["nc"]


_BORDER = [0, 1, 5, OUT_HW - 6, OUT_HW - 2, OUT_HW - 1]


def kernel(x: np.ndarray, rand_vals: np.ndarray, **run_kwargs) -> np.ndarray:
    b, c, h, w = x.shape
    assert (b, c, h, w) == (4, 256, 64, 64)
    n_total = b * c

    keep = np.asarray(rand_vals) > RATE
    keep[S2::S, S2::S] = True
    keep[_BORDER, :] = False
    keep[:, _BORDER] = False
    m01 = keep.astype(ml_dtypes.bfloat16).reshape(GROUPS, G_F)

    scale = np.float32(1.0) / np.float32(1.0 - RATE)
    x_bf = (np.asarray(x).reshape(n_total, h * w) * scale).astype(ml_dtypes.bfloat16)
    per_core = n_total // N_CORES
    in_maps = [
        {
            "x": np.ascontiguousarray(x_bf[k * per_core : (k + 1) * per_core]),
            "mask": m01,
        }
        for k in range(N_CORES)
    ]

    nc = _get_nc()
    res = run_bass_kernel_spmd(nc, in_maps, core_ids=list(range(N_CORES)), **run_kwargs)
    out = np.concatenate([r["out"] for r in res.results], axis=0)
    _CACHE["last_results"] = res
    return out.astype(np.float32).reshape(b, c, OUT_HW, OUT_HW)


# revision 15
# speedup vs baseline: 1.3219x; 1.3219x over previous
"""Trainium2 Bass kernel for Expansion + CPSDropout (bf16 output).

Computes, for x[4,256,64,64] f32 and rand_vals[320,320] f32:
    out[b,c,5i+p,5j+q] = xpad[b,c,i+p,j+q] * M[5i+p,5j+q]
    M = (rand_vals > 0.25, forced True at [2::5,2::5]) * (4/3)

Strategy (8 cores, data parallel over the 1024 (b,c) channels, 128/core):
  - host: x*(4/3) -> bf16; binary mask {0,1} bf16 with zeros at border
    positions whose source reads fall in the zero padding.
  - device: x staged into guard-padded xg[128, 4360]; each i-tile's 1600
    outputs come from ONE DVE tensor_tensor with input AP
    out[a,j,q] = xg[64*(i+a) + j + q] * mask[a*320+5j+q].
  - mask replication across partitions via PE broadcast (ones^T @ mask)
    into [128,2048] PSUM tiles (512-col matmuls, bank aligned).
    per group of 4 i-tiles, 3 are evacuated PSUM->SBUF bf16 by ScalarE
    (DVE multiplies from SBUF, faster mode) and 1 is multiplied by the
    DVE straight from PSUM — balancing ScalarE/DVE against the PE-paced
    2-deep PSUM ring.
  - output bf16, host upcasts to f32 (rel err ~2e-3 << 2e-2 gate).
"""

import numpy as np
import ml_dtypes

import concourse.bass as bass
import concourse.bacc as bacc
import concourse.mybir as mybir
import concourse.tile as tile
from concourse.bass_utils import run_bass_kernel_spmd

P = 128
N_CORES = 8
H = W = 64
S = 5
S2 = S // 2
OUT_HW = H * S            # 320
OUT_ELEMS = OUT_HW * OUT_HW
RATE = 0.25

XG_F = 68 * W + 8         # 4360
XG_X0 = 2 * W + 2         # 130

I_PER_G = 4               # i-tiles per store group
GROUPS = H // I_PER_G     # 8
TILE_F = S * OUT_HW       # 1600
G_F = I_PER_G * TILE_F    # 12800

TILE_PATTERN = "AAAC"  # per-group i-tile paths (3 evac'd, 1 PSUM-direct)

_CACHE = {}


def _build_nc():
    assert len(TILE_PATTERN) == I_PER_G
    nc = bacc.Bacc("TRN2", target_bir_lowering=False)
    x_t = nc.dram_tensor("x", [P, H * W], mybir.dt.bfloat16, kind="ExternalInput")
    m_t = nc.dram_tensor("mask", [GROUPS, G_F], mybir.dt.bfloat16, kind="ExternalInput")
    o_t = nc.dram_tensor("out", [P, OUT_ELEMS], mybir.dt.bfloat16,
                         kind="ExternalOutput")

    with tile.TileContext(nc) as tc:
        with (
            tc.tile_pool(name="const", bufs=1) as constp,
            tc.tile_pool(name="xbuf", bufs=1) as xbufp,
            tc.tile_pool(name="mstage", bufs=4) as mstp,
            tc.tile_pool(name="msb", bufs=2) as msbp,
            tc.tile_pool(name="obuf", bufs=3) as obufp,
            tc.tile_pool(name="mpsum", bufs=2, space="PSUM") as psump,
        ):
            ones_bf = constp.tile([1, P], mybir.dt.bfloat16)
            nc.vector.memset(ones_bf[:], 1.0)

            xg = xbufp.tile([P, XG_F], mybir.dt.bfloat16)
            # only the guard strips outside x need to be finite (borders are
            # killed by host-side mask zeros); keep ScalarE free
            nc.vector.memset(xg[:, 0:XG_X0], 0.0)
            nc.vector.memset(xg[:, XG_X0 + H * W : XG_F], 0.0)
            nc.gpsimd.dma_start(out=xg[:, XG_X0 : XG_X0 + H * W], in_=x_t[:])

            xg_ap = xg[:]
            xg_pdim = list(xg_ap.ap[0])
            TT_DIMS = [[OUT_HW, S], [S, H], [1, S]]          # (a, j, q) dense
            TT_DIMS2 = [[2 * OUT_HW, S], [2 * S, H], [2, S]]  # stride-2 bf16 view

            def tt(i, out_tile, out_off, in1_tensor, in1_offset, in1_pdim,
                   dims=TT_DIMS):
                in0 = bass.AP(
                    tensor=xg_ap.tensor,
                    offset=xg_ap.offset + i * W,
                    ap=[xg_pdim, [W, S], [1, H], [1, S]],
                )
                o1 = bass.AP(
                    tensor=out_tile.tensor,
                    offset=out_tile.offset + out_off,
                    ap=[list(out_tile.ap[0])] + TT_DIMS,
                )
                i1 = bass.AP(
                    tensor=in1_tensor, offset=in1_offset,
                    ap=[in1_pdim] + dims,
                )
                nc.vector.tensor_tensor(
                    out=o1, in0=in0, in1=i1, op=mybir.AluOpType.mult
                )

            for g in range(GROUPS):
                obuf = obufp.tile([P, G_F], mybir.dt.bfloat16)
                mst = mstp.tile([1, G_F], mybir.dt.bfloat16)
                nc.gpsimd.dma_start(out=mst[:], in_=m_t[g : g + 1, :])

                msb = msbp.tile([P, G_F], mybir.dt.bfloat16)
                msb_ap = msb[:]
                for u in range(I_PER_G):
                    path = TILE_PATTERN[u]
                    i = g * I_PER_G + u
                    ps = psump.tile([P, TILE_F], mybir.dt.float32)
                    for j0, j1 in ((0, 512), (512, 1024), (1024, 1536),
                                   (1536, 1600)):
                        nc.tensor.matmul(
                            ps[:, j0:j1],
                            ones_bf[:],
                            mst[0:1, u * TILE_F + j0 : u * TILE_F + j1],
                            start=True, stop=True,
                        )
                    if path == "A":
                        nc.scalar.copy(
                            out=msb[:, u * TILE_F : (u + 1) * TILE_F], in_=ps[:]
                        )
                        tt(i, obuf[:], u * TILE_F,
                           msb_ap.tensor, msb_ap.offset + u * TILE_F,
                           list(msb_ap.ap[0]))
                    else:
                        ps_ap = ps[:]
                        tt(i, obuf[:], u * TILE_F,
                           ps_ap.tensor, ps_ap.offset, list(ps_ap.ap[0]))
                nc.sync.dma_start(out=o_t[:, g * G_F : (g + 1) * G_F], in_=obuf[:])
    nc.compile()
    return nc


def _get_nc():
    if "nc" not in _CACHE:
        _CACHE["nc"] = _build_nc()
    return _CACHE["nc"]


_BORDER = [0, 1, 5, OUT_HW - 6, OUT_HW - 2, OUT_HW - 1]


def kernel(x: np.ndarray, rand_vals: np.ndarray, **run_kwargs) -> np.ndarray:
    b, c, h, w = x.shape
    assert (b, c, h, w) == (4, 256, 64, 64)
    n_total = b * c

    keep = np.asarray(rand_vals) > RATE
    keep[S2::S, S2::S] = True
    keep[_BORDER, :] = False
    keep[:, _BORDER] = False
    m01 = keep.astype(ml_dtypes.bfloat16).reshape(GROUPS, G_F)

    scale = np.float32(1.0) / np.float32(1.0 - RATE)
    x_bf = (np.asarray(x).reshape(n_total, h * w) * scale).astype(ml_dtypes.bfloat16)
    per_core = n_total // N_CORES
    in_maps = [
        {
            "x": np.ascontiguousarray(x_bf[k * per_core : (k + 1) * per_core]),
            "mask": m01,
        }
        for k in range(N_CORES)
    ]

    nc = _get_nc()
    res = run_bass_kernel_spmd(nc, in_maps, core_ids=list(range(N_CORES)), **run_kwargs)
    out = np.concatenate([r["out"] for r in res.results], axis=0)
    _CACHE["last_results"] = res
    return out.astype(np.float32).reshape(b, c, OUT_HW, OUT_HW)


# revision 16
# speedup vs baseline: 1.3365x; 1.0111x over previous
"""Trainium2 Bass kernel for Expansion + CPSDropout (bf16 output).

Computes, for x[4,256,64,64] f32 and rand_vals[320,320] f32:
    out[b,c,5i+p,5j+q] = xpad[b,c,i+p,j+q] * M[5i+p,5j+q]
    M = (rand_vals > 0.25, forced True at [2::5,2::5]) * (4/3)

Strategy (8 cores, data parallel over the 1024 (b,c) channels, 128/core):
  - host: x*(4/3) -> bf16; binary mask {0,1} bf16 with zeros at border
    positions whose source reads fall in the zero padding.
  - device: x staged into guard-padded xg[128, 4360]; each i-tile's 1600
    outputs come from ONE DVE tensor_tensor with input AP
    out[a,j,q] = xg[64*(i+a) + j + q] * mask[a*320+5j+q].
  - mask replication across partitions via PE broadcast (ones^T @ mask)
    into [128,2048] PSUM tiles (512-col matmuls, bank aligned).
    per group of 4 i-tiles, 3 are evacuated PSUM->SBUF bf16 by ScalarE
    (DVE multiplies from SBUF, faster mode) and 1 is multiplied by the
    DVE straight from PSUM — balancing ScalarE/DVE against the PE-paced
    2-deep PSUM ring.
  - output bf16, host upcasts to f32 (rel err ~2e-3 << 2e-2 gate).
"""

import numpy as np
import ml_dtypes

import concourse.bass as bass
import concourse.bacc as bacc
import concourse.mybir as mybir
import concourse.tile as tile
from concourse.bass_utils import run_bass_kernel_spmd

P = 128
N_CORES = 8
H = W = 64
S = 5
S2 = S // 2
OUT_HW = H * S            # 320
OUT_ELEMS = OUT_HW * OUT_HW
RATE = 0.25

XG_F = 68 * W + 8         # 4360
XG_X0 = 2 * W + 2         # 130

I_PER_G = 4               # i-tiles per store group
GROUPS = H // I_PER_G     # 8
TILE_F = S * OUT_HW       # 1600
G_F = I_PER_G * TILE_F    # 12800

TILE_PATTERN = "AAAC"  # per-group i-tile paths (3 evac'd, 1 PSUM-direct)

_CACHE = {}


def _build_nc():
    assert len(TILE_PATTERN) == I_PER_G
    nc = bacc.Bacc("TRN2", target_bir_lowering=False)
    x_t = nc.dram_tensor("x", [P, H * W], mybir.dt.bfloat16, kind="ExternalInput")
    m_t = nc.dram_tensor("mask", [GROUPS, G_F], mybir.dt.bfloat16, kind="ExternalInput")
    o_t = nc.dram_tensor("out", [P, OUT_ELEMS], mybir.dt.bfloat16,
                         kind="ExternalOutput")

    with tile.TileContext(nc) as tc:
        with (
            tc.tile_pool(name="const", bufs=1) as constp,
            tc.tile_pool(name="xbuf", bufs=1) as xbufp,
            tc.tile_pool(name="mstage", bufs=4) as mstp,
            tc.tile_pool(name="msb", bufs=2) as msbp,
            tc.tile_pool(name="obuf", bufs=3) as obufp,
            tc.tile_pool(name="mpsum", bufs=2, space="PSUM") as psump,
        ):
            ones_bf = constp.tile([1, P], mybir.dt.bfloat16)
            nc.vector.memset(ones_bf[:], 1.0)

            xg = xbufp.tile([P, XG_F], mybir.dt.bfloat16)
            # only the guard strips outside x need to be finite (borders are
            # killed by host-side mask zeros); keep ScalarE free
            nc.vector.memset(xg[:, 0:XG_X0], 0.0)
            nc.vector.memset(xg[:, XG_X0 + H * W : XG_F], 0.0)
            nc.gpsimd.dma_start(out=xg[:, XG_X0 : XG_X0 + H * W], in_=x_t[:])

            xg_ap = xg[:]
            xg_pdim = list(xg_ap.ap[0])
            TT_DIMS = [[OUT_HW, S], [S, H], [1, S]]          # (a, j, q) dense
            TT_DIMS2 = [[2 * OUT_HW, S], [2 * S, H], [2, S]]  # stride-2 bf16 view

            def tt(i, out_tile, out_off, in1_tensor, in1_offset, in1_pdim,
                   dims=TT_DIMS):
                in0 = bass.AP(
                    tensor=xg_ap.tensor,
                    offset=xg_ap.offset + i * W,
                    ap=[xg_pdim, [W, S], [1, H], [1, S]],
                )
                o1 = bass.AP(
                    tensor=out_tile.tensor,
                    offset=out_tile.offset + out_off,
                    ap=[list(out_tile.ap[0])] + TT_DIMS,
                )
                i1 = bass.AP(
                    tensor=in1_tensor, offset=in1_offset,
                    ap=[in1_pdim] + dims,
                )
                nc.vector.tensor_tensor(
                    out=o1, in0=in0, in1=i1, op=mybir.AluOpType.mult
                )

            for g in range(GROUPS):
                obuf = obufp.tile([P, G_F], mybir.dt.bfloat16)
                mst = mstp.tile([1, G_F], mybir.dt.bfloat16)
                nc.gpsimd.dma_start(out=mst[:], in_=m_t[g : g + 1, :])

                msb = msbp.tile([P, G_F], mybir.dt.bfloat16)
                msb_ap = msb[:]
                for u in range(I_PER_G):
                    path = TILE_PATTERN[u]
                    i = g * I_PER_G + u
                    ps = psump.tile([P, TILE_F], mybir.dt.float32)
                    for j0, j1 in ((0, 512), (512, 1024), (1024, 1536),
                                   (1536, 1600)):
                        nc.tensor.matmul(
                            ps[:, j0:j1],
                            ones_bf[:],
                            mst[0:1, u * TILE_F + j0 : u * TILE_F + j1],
                            start=True, stop=True,
                        )
                    if path == "A":
                        nc.scalar.copy(
                            out=msb[:, u * TILE_F : (u + 1) * TILE_F], in_=ps[:]
                        )
                        tt(i, obuf[:], u * TILE_F,
                           msb_ap.tensor, msb_ap.offset + u * TILE_F,
                           list(msb_ap.ap[0]))
                    else:
                        ps_ap = ps[:]
                        tt(i, obuf[:], u * TILE_F,
                           ps_ap.tensor, ps_ap.offset, list(ps_ap.ap[0]))
                if g == GROUPS - 1:
                    # split the final store per i-tile so the drain overlaps
                    # the last TTs instead of trailing the kernel
                    for u in range(I_PER_G):
                        i = g * I_PER_G + u
                        nc.sync.dma_start(
                            out=o_t[:, i * TILE_F : (i + 1) * TILE_F],
                            in_=obuf[:, u * TILE_F : (u + 1) * TILE_F],
                        )
                else:
                    nc.sync.dma_start(
                        out=o_t[:, g * G_F : (g + 1) * G_F], in_=obuf[:]
                    )
    nc.compile()
    return nc


def _get_nc():
    if "nc" not in _CACHE:
        _CACHE["nc"] = _build_nc()
    return _CACHE["nc"]


_BORDER = [0, 1, 5, OUT_HW - 6, OUT_HW - 2, OUT_HW - 1]


def kernel(x: np.ndarray, rand_vals: np.ndarray, **run_kwargs) -> np.ndarray:
    b, c, h, w = x.shape
    assert (b, c, h, w) == (4, 256, 64, 64)
    n_total = b * c

    keep = np.asarray(rand_vals) > RATE
    keep[S2::S, S2::S] = True
    keep[_BORDER, :] = False
    keep[:, _BORDER] = False
    m01 = keep.astype(ml_dtypes.bfloat16).reshape(GROUPS, G_F)

    scale = np.float32(1.0) / np.float32(1.0 - RATE)
    x_bf = (np.asarray(x).reshape(n_total, h * w) * scale).astype(ml_dtypes.bfloat16)
    per_core = n_total // N_CORES
    in_maps = [
        {
            "x": np.ascontiguousarray(x_bf[k * per_core : (k + 1) * per_core]),
            "mask": m01,
        }
        for k in range(N_CORES)
    ]

    nc = _get_nc()
    res = run_bass_kernel_spmd(nc, in_maps, core_ids=list(range(N_CORES)), **run_kwargs)
    out = np.concatenate([r["out"] for r in res.results], axis=0)
    _CACHE["last_results"] = res
    return out.astype(np.float32).reshape(b, c, OUT_HW, OUT_HW)
